# revision 18
# baseline (speedup 1.0000x reference)
"""Trainium2 Bass kernel v4: scatter-gather edges via gpsimd local_scatter.

Layout: 8 cores shard by dst (12500 nodes each). Per core, 128 SBUF
partitions = (src-group sigma in [0,32) global, 3125 nodes each) x
(dst-quarter h in [0,4) of the core's shard, 3136-padded). Partition
p = sigma*4 + h holds its edges sorted by dst-local.

Per-edge src values (psrc): local_scatter with data = the group's node
values repeated R times (R = max edge multiplicity per (partition, src));
idx[(r, j)] = stream slot of the r-th edge of src j, -1 if none. Replaces
the ap_gather passes (~26ns/idx) with ~2ns/idx scatters.

Per-edge dst values (pv): impulse local_scatter at run starts + masked
scan broadcast (as before). Per-dst segment sums: local_scatter run-end
scan values to dst slots + one-hot PE matmul collapse over sigma.
"""
import numpy as np
import ml_dtypes

N_NODES = 100000

# ===================== tile drain workaround =====================
import concourse.tile as tile
from concourse import mybir
from bass_rust import ScopedClock


def _patched_drain_and_barrier(self, tick_clock, wait_clock):
    nc = self.nc
    probe = nc.sync.nop()
    wait_clock.add_sem_waits(probe.ins, ScopedClock({None: tick_clock.global_clock}))
    si = probe.ins.sync_info
    waits = list(si.on_wait) if si is not None else []
    if si is not None:
        si.on_wait = waits[:1]
    for w in waits[1:]:
        nop = nc.sync.nop()
        nop.ins.sync_info = mybir.SyncInfo(on_wait=[w], on_update=[])
    nc.sync.drain()
    nc.all_engine_barrier()
    popped = nc._tile_sem_poison_stack.pop()
    assert popped is self._sem_poison
    nc.clear_and_free_semaphores(list(self.sems.allocated().values()))
    nc.all_engine_barrier()


def install():
    tile.TileContext._drain_and_barrier = _patched_drain_and_barrier


install()

from contextlib import ExitStack
import concourse.bass as bass
import concourse.bacc as bacc

F32 = mybir.dt.float32
I16 = mybir.dt.int16
BF16 = mybir.dt.bfloat16
AF = mybir.ActivationFunctionType
ALU = mybir.AluOpType
AX = mybir.AxisListType

NSIG = 32          # global src groups
GS = 3125          # nodes per group
GSP = 3126         # padded (even) data cols per group
NH = 4             # dst quarters per core
QS = 3136          # padded quarter size (12544/4)
NCH = 5            # round chunks per half


def prep(edge_index: np.ndarray, N: int):
    n_cores = 8
    assert N == N_NODES and N % n_cores == 0
    Nsh = N // n_cores            # 12500
    Nshp = 12544
    W = Nshp // 128               # 98

    src = edge_index[0].astype(np.int64)
    dst = edge_index[1].astype(np.int64)
    core = dst // Nsh
    dstl = dst - core * Nsh
    sig = src // GS
    h = dstl // QS
    p = sig * 4 + h
    key = (core * 128 + p) * Nsh + dstl
    order = np.argsort(key, kind="stable")
    src_s = src[order]
    dstl_s = dstl[order].astype(np.int32)
    cp = (core * 128 + p)[order]

    cp_starts = np.searchsorted(cp, np.arange(n_cores * 128))
    cp_ends = np.searchsorted(cp, np.arange(n_cores * 128), side="right")
    counts = cp_ends - cp_starts
    L = int(counts.max())
    L = -(-L // 16) * 16
    LH = L // 2
    assert LH <= 2046 and LH % 2 == 0, f"LH={LH}"

    def occ_of(j):
        """occurrence index among equal values, preserving order"""
        n = len(j)
        if n == 0:
            return np.zeros(0, dtype=np.int64)
        o2 = np.argsort(j, kind="stable")
        js = j[o2]
        newg = np.empty(n, dtype=bool)
        newg[0] = True
        np.not_equal(js[1:], js[:-1], out=newg[1:])
        start_pos = np.maximum.accumulate(np.where(newg, np.arange(n), 0))
        occ = np.empty(n, dtype=np.int64)
        occ[o2] = np.arange(n) - start_pos
        return occ

    # first pass: max multiplicity R, counted PER STREAM-HALF (each half
    # is its own scatter call, so only within-half uniqueness matters)
    R = 1
    occs = []
    for ci in range(n_cores * 128):
        a, b = cp_starts[ci], cp_ends[ci]
        j = (src_s[a:b] % GS).astype(np.int64)
        n = b - a
        mA = np.arange(n) < LH
        occA = occ_of(j[mA])
        occB = occ_of(j[~mA])
        if len(occA):
            R = max(R, int(occA.max()) + 1)
        if len(occB):
            R = max(R, int(occB.max()) + 1)
        occs.append((occA, occB))
    RH = 2
    Rp = -(-R // RH) * RH
    SGW = Rp * GSP

    cores = []
    for c in range(n_cores):
        runstart = np.ones((128, L), dtype=np.float32)
        impA = np.full((128, QS), -1, dtype=np.int16)
        impB = np.full((128, QS), -1, dtype=np.int16)
        bidx0 = np.full((128, L), -1, dtype=np.int16)
        bidx1 = np.full((128, L), -1, dtype=np.int16)
        sgA = np.full((128, SGW), -1, dtype=np.int16)
        sgB = np.full((128, SGW), -1, dtype=np.int16)

        for pi in range(128):
            ci = c * 128 + pi
            a, b = cp_starts[ci], cp_ends[ci]
            n = b - a
            if n == 0:
                continue
            hq = pi % 4
            dd = dstl_s[a:b]
            jj = (src_s[a:b] % GS).astype(np.int64)
            occA, occB = occs[ci]
            slots = np.arange(n)
            inA = slots < LH
            posA = occA * GSP + jj[inA]
            posB = occB * GSP + jj[~inA]
            sgA[pi][posA] = slots[inA].astype(np.int16)
            sgB[pi][posB] = (slots[~inA] - LH).astype(np.int16)

            newrun = np.empty(n, dtype=bool)
            newrun[0] = True
            np.not_equal(dd[1:], dd[:-1], out=newrun[1:])
            rs = np.flatnonzero(newrun)
            runstart[pi, rs] = 0.0
            run_dst = dd[rs] - hq * QS          # dst pos within quarter
            run_ends = np.append(rs[1:], n) - 1
            lo = rs < LH
            impA[pi, run_dst[lo]] = rs[lo].astype(np.int16)
            impB[pi, run_dst[~lo]] = (rs[~lo] - LH).astype(np.int16)
            sub0 = run_dst < (QS // 2)
            bidx0[pi, run_ends[sub0]] = run_dst[sub0].astype(np.int16)
            bidx1[pi, run_ends[~sub0]] = (run_dst[~sub0] -
                                          QS // 2).astype(np.int16)

        cores.append(dict(runstart=runstart, impA=impA, impB=impB,
                          bidx0=bidx0, bidx1=bidx1, sgA=sgA, sgB=sgB))

    w4 = np.zeros((128, 4), dtype=np.float32)
    w4[np.arange(128), np.arange(128) % 4] = 1.0

    return dict(N=N, Nsh=Nsh, Nshp=Nshp, W=W, L=L, LH=LH, R=R, RH=RH,
                Rp=Rp, SGW=SGW, cores=cores, w4=w4)


def build(pp, no_collective=False):
    N, Nsh, Nshp, W = pp["N"], pp["Nsh"], pp["Nshp"], pp["W"]
    L, LH, RH, Rp, SGW = pp["L"], pp["LH"], pp["RH"], pp["Rp"], pp["SGW"]
    R = pp["R"]
    NF = -(-N // 128)
    PF = (8 * Nshp) // 128
    CW = RH * GSP                   # idx cols per round-chunk call

    nc = bacc.Bacc("TRN2", target_bir_lowering=False, debug=False,
                   num_devices=8)

    def din(name, shape, dt=F32):
        return nc.dram_tensor(name, shape, dt, kind="ExternalInput")

    x_full = din("x_full", [1, 128 * NF])
    xb_full = din("xb_full", [1, NSIG * GS], BF16)
    x_shard = din("x_shard", [1, Nshp])
    x_shardb = din("x_shardb", [1, Nshp], BF16)
    W1 = din("W1", [1, 20])
    a_src1 = din("a_src1", [1, 20])
    a_dst1 = din("a_dst1", [1, 20])
    W2T = din("W2T", [1, 400])
    a_src2 = din("a_src2", [1, 20])
    a_dst2 = din("a_dst2", [1, 20])
    b2 = din("b2", [1, 20])
    Wl = din("Wl", [1, 20])
    bl = din("bl", [1, 1])
    runst = din("runstart", [128, L], BF16)
    impAd = din("impA", [128, QS], I16)
    impBd = din("impB", [128, QS], I16)
    bidx0d = din("bidx0", [128, L], I16)
    bidx1d = din("bidx1", [128, L], I16)
    sgAd = din("sgA", [128, SGW], I16)
    sgBd = din("sgB", [128, SGW], I16)
    w4d = din("w4", [128, 4])

    y_out = nc.dram_tensor("y", [1, Nshp], F32, kind="ExternalOutput")

    p_localb = nc.dram_tensor("p_localb", [1, Nshp], BF16)
    p_fullb = nc.dram_tensor("p_fullb", [1, 8 * Nshp], BF16,
                             addr_space="Shared")
    v2_local = nc.dram_tensor("v2_local", [1, Nshp], BF16)
    rt128 = nc.dram_tensor("rt128", [1, 128], F32)
    rt1 = nc.dram_tensor("rt1", [1, 1], F32)
    sums_all = nc.dram_tensor("sums_all", [1, 5 * Nshp], F32)

    with tile.TileContext(nc) as tc, ExitStack() as ctx:
        consts = ctx.enter_context(tc.tile_pool(name="consts", bufs=1))
        smalls = ctx.enter_context(tc.tile_pool(name="smalls", bufs=2))
        strm = ctx.enter_context(tc.tile_pool(name="strm", bufs=5))
        sgp = ctx.enter_context(tc.tile_pool(name="sg", bufs=2))
        datp = ctx.enter_context(tc.tile_pool(name="dat", bufs=1))
        halfp = ctx.enter_context(tc.tile_pool(name="half", bufs=4))
        impp = ctx.enter_context(tc.tile_pool(name="imp", bufs=1))
        bnd = ctx.enter_context(tc.tile_pool(name="bnd", bufs=3))
        nodep = ctx.enter_context(tc.tile_pool(name="node", bufs=1))
        psp = ctx.enter_context(tc.tile_pool(name="ps", bufs=2, space="PSUM"))

        def S(name, dt=BF16):
            return strm.tile([128, L], dt, tag="s", name=name, bufs=5)

        def bcast(dram_ap, n, name):
            t = consts.tile([128, n], F32, name=name)
            nc.sync.dma_start(t[:], dram_ap.partition_broadcast(128))
            return t

        def rsc(t, name, op=ALU.add):
            out = consts.tile([128, 1], F32, name=name)
            nc.vector.tensor_reduce(out[:], t[:], AX.X, op)
            return out

        def tmul(a, b, name, pool=None):
            out = (pool or smalls).tile(list(a.shape), F32, name=name)
            nc.vector.tensor_tensor(out=out[:], in0=a[:], in1=b[:],
                                    op=ALU.mult)
            return out

        # ------------- constants -------------
        wb = bcast(W1.ap(), 20, "wb")
        a1s = bcast(a_src1.ap(), 20, "a1s")
        a1d = bcast(a_dst1.ap(), 20, "a1d")
        a2s = bcast(a_src2.ap(), 20, "a2s")
        a2d = bcast(a_dst2.ap(), 20, "a2d")
        b2t = bcast(b2.ap(), 20, "b2t")
        wlt = bcast(Wl.ap(), 20, "wlt")
        blt = bcast(bl.ap(), 1, "blt")
        w2t = bcast(W2T.ap(), 400, "w2t")

        c1 = rsc(tmul(wb, a1s, "c1m"), "c1")
        c2 = rsc(tmul(wb, a1d, "c2m"), "c2")
        c1c2 = consts.tile([128, 1], F32, name="c1c2")
        nc.vector.tensor_tensor(out=c1c2[:], in0=c1[:], in1=c2[:], op=ALU.add)

        wp = consts.tile([128, 20], F32, name="wp")
        nc.scalar.activation(wp[:], wb[:], AF.Relu)
        wm = consts.tile([128, 20], F32, name="wm")
        nc.scalar.activation(wm[:], wb[:], AF.Relu, scale=-1.0)

        def qvec(wv, name):
            t = smalls.tile([128, 400], F32, name=name + "_t", tag="q400",
                            bufs=1)
            nc.vector.tensor_tensor(
                out=t[:], in0=w2t[:],
                in1=wv[:].unsqueeze(1).broadcast_to([128, 20, 20]),
                op=ALU.mult)
            out = consts.tile([128, 20], F32, name=name)
            nc.vector.tensor_reduce(
                out[:], t[:].rearrange("p (j k) -> p j k", j=20), AX.X,
                ALU.add)
            return out

        qp = qvec(wp, "qp")
        qm = qvec(wm, "qm")
        A2 = rsc(tmul(qp, a2s, "A2m"), "A2")
        B2 = rsc(tmul(qm, a2s, "B2m"), "B2")
        C2 = rsc(tmul(qp, a2d, "C2m"), "C2")
        D2 = rsc(tmul(qm, a2d, "D2m"), "D2")
        A2B2 = consts.tile([128, 1], F32, name="A2B2")
        nc.vector.tensor_tensor(out=A2B2[:], in0=A2[:], in1=B2[:], op=ALU.add)
        nB2 = consts.tile([128, 1], F32, name="nB2")
        nc.scalar.mul(nB2[:], B2[:], -1.0)
        C2D2 = consts.tile([128, 1], F32, name="C2D2")
        nc.vector.tensor_tensor(out=C2D2[:], in0=C2[:], in1=D2[:], op=ALU.add)
        nD2 = consts.tile([128, 1], F32, name="nD2")
        nc.scalar.mul(nD2[:], D2[:], -1.0)

        w4b = consts.tile([128, 4], BF16, name="w4b")
        w4s = consts.tile([128, 4], F32, name="w4s")
        nc.sync.dma_start(w4s[:], w4d.ap())
        nc.scalar.copy(w4b[:], w4s[:])

        def cross_max(tin, name):
            m = smalls.tile([128, 1], F32, name=name + "_m")
            nc.vector.tensor_reduce(m[:], tin[:], AX.X, ALU.max)
            nc.sync.dma_start(rt128.ap(), m[:])
            row = smalls.tile([1, 128], F32, name=name + "_row")
            nc.sync.dma_start(row[:], rt128.ap())
            m1 = smalls.tile([1, 1], F32, name=name + "_m1")
            nc.vector.tensor_reduce(m1[:], row[:], AX.X, ALU.max)
            nc.sync.dma_start(rt1.ap(), m1[:])
            mb = consts.tile([128, 1], F32, name=name)
            nc.sync.dma_start(mb[:], rt1.ap().partition_broadcast(128))
            return mb

        def relu_pair_max(dram_ap, ncols, name, dt=F32):
            big = nodep.tile([128, ncols], dt, tag="gmax", name=name + "_big",
                             bufs=1)
            nc.sync.dma_start(big[:], dram_ap)
            outs = []
            for i, sgn in enumerate((1.0, -1.0)):
                r = nodep.tile([128, ncols], F32, tag="gmaxr",
                               name=f"{name}_r{i}", bufs=1)
                nc.scalar.activation(r[:], big[:], AF.Relu, scale=sgn)
                outs.append(cross_max(r, f"{name}{i}"))
            return outs

        def sc1(name):
            return consts.tile([128, 1], F32, name=name)

        def lrelu_neg(t, name):
            o = sc1(name + "_lr")
            nc.vector.scalar_tensor_tensor(out=o[:], in0=t[:], scalar=0.2,
                                           in1=t[:], op0=ALU.mult,
                                           op1=ALU.max)
            o2 = sc1(name)
            nc.scalar.mul(o2[:], o[:], -1.0)
            return o2

        mxp, mxm = relu_pair_max(x_full.ap(), NF, "mx")

        def ub_exact(cc, name):
            t1 = tmul(cc, mxp, name + "_1")
            ncc = smalls.tile([128, 1], F32, name=name + "_n")
            nc.scalar.mul(ncc[:], cc[:], -1.0)
            t2 = tmul(ncc, mxm, name + "_2")
            o = smalls.tile([128, 1], F32, name=name)
            nc.vector.tensor_tensor(out=o[:], in0=t1[:], in1=t2[:], op=ALU.max)
            return o

        ub1 = smalls.tile([128, 1], F32, name="ub1")
        nc.vector.tensor_tensor(out=ub1[:], in0=ub_exact(c1, "ubu1")[:],
                                in1=ub_exact(c2, "ubv1")[:], op=ALU.add)
        gneg1 = lrelu_neg(ub1, "gneg1")

        # ------------- shared tiles -------------
        runstart_t = nodep.tile([128, L], BF16, name="runstart_t")
        nc.sync.dma_start(runstart_t[:], runst.ap())
        bidx0_t = nodep.tile([128, L], I16, name="bidx0_t")
        nc.sync.dma_start(bidx0_t[:], bidx0d.ap())
        bidx1_t = nodep.tile([128, L], I16, name="bidx1_t")
        nc.sync.dma_start(bidx1_t[:], bidx1d.ap())
        impA_t = nodep.tile([128, QS], I16, name="impA_t")
        nc.sync.dma_start(impA_t[:], impAd.ap())
        impB_t = nodep.tile([128, QS], I16, name="impB_t")
        nc.sync.dma_start(impB_t[:], impBd.ap())

        def sg_pass(base, name):
            """psrc stream via repeated-table local_scatter rounds.
            base: [128, GSP] bf16 group-table tile.
            """
            rep = datp.tile([128, CW], BF16, tag="rep", name=name + "_rep",
                            bufs=1)
            for r in range(RH):
                nc.scalar.copy(rep[:, r * GSP:(r + 1) * GSP], base[:])
            psrc = strm.tile([128, L], BF16, tag="pk", name=name, bufs=1)
            nch_eff = -(-R // RH)
            for off, sgd in ((0, sgAd), (LH, sgBd)):
                acc = None
                for ch in range(nch_eff):
                    # trim the all-padding rounds in the last chunk
                    cw = min(CW, (R - ch * RH) * GSP)
                    it = sgp.tile([128, CW], I16, tag="sgi",
                                  name=f"{name}_i{off}_{ch}", bufs=2)
                    nc.sync.dma_start(
                        it[:, 0:cw], sgd.ap()[:, ch * CW:ch * CW + cw])
                    o = halfp.tile([128, LH], BF16, tag="ho",
                                   name=f"{name}_o{off}_{ch}", bufs=2)
                    nc.gpsimd.local_scatter(o[:], rep[:], it[:, 0:cw],
                                            channels=128, num_elems=LH,
                                            num_idxs=cw)
                    last = ch == nch_eff - 1
                    tgt = psrc[:, off:off + LH]
                    if acc is None:
                        if last:
                            nc.scalar.copy(tgt, o[:])
                        else:
                            acc = halfp.tile([128, LH], BF16, tag="hacc",
                                             name=f"{name}_a{off}", bufs=2)
                            nc.scalar.copy(acc[:], o[:])
                    else:
                        nc.vector.tensor_tensor(
                            out=tgt if last else acc[:], in0=acc[:],
                            in1=o[:], op=ALU.add)
            return psrc

        def impulse_bcast(src_dram, name):
            """[1, Nshp] bf16 DRAM -> per-partition dst-quarter values
            broadcast over runs: bf16 [128, L] stream."""
            stageb = impp.tile([128, QS], BF16, tag="impb",
                               name=name + "_sb", bufs=2)
            src_ap = src_dram.ap().rearrange("a (h j) -> (a h) j", h=4)
            for sg in range(NSIG):
                (nc.scalar if sg % 2 else nc.sync).dma_start(
                    stageb[4 * sg:4 * sg + 4, :], src_ap)
            imp = strm.tile([128, L], BF16, tag="imp", name=name + "_imp",
                            bufs=1)
            nc.gpsimd.local_scatter(imp[:, 0:LH], stageb[:], impA_t[:],
                                    channels=128, num_elems=LH, num_idxs=QS)
            nc.gpsimd.local_scatter(imp[:, LH:L], stageb[:], impB_t[:],
                                    channels=128, num_elems=L - LH,
                                    num_idxs=QS)
            out = S(name)
            nc.vector.tensor_tensor_scan(
                out[:], runstart_t[:], imp[:], 0.0, ALU.mult, ALU.add)
            return out

        def seg_scan(data, name):
            s = S(name)
            nc.vector.tensor_tensor_scan(
                s[:], runstart_t[:], data[:], 0.0, ALU.mult, ALU.add)
            return s

        def bscatter(sct, si, name):
            """Run-end extraction into dst-quarter slots + sigma-collapse."""
            win = sums_all.ap()[:, si * Nshp:(si + 1) * Nshp].rearrange(
                "a (h j) -> (a h) j", h=4)
            for sub, bt in ((0, bidx0_t), (1, bidx1_t)):
                out = bnd.tile([128, QS // 2], BF16, tag="bs",
                               name=f"bs_{name}_{sub}", bufs=3)
                nc.gpsimd.local_scatter(out[:], sct[:], bt[:],
                                        channels=128, num_elems=QS // 2,
                                        num_idxs=L)
                for k0 in range(0, QS // 2, 512):
                    kn = min(512, QS // 2 - k0)
                    ps = psp.tile([4, kn], F32, tag="ps",
                                  name=f"ps_{name}_{sub}_{k0}")
                    nc.tensor.matmul(ps[:], w4b[:], out[:, k0:k0 + kn],
                                     start=True, stop=True)
                    ev = bnd.tile([4, kn], F32, tag="ev",
                                  name=f"ev_{name}_{sub}_{k0}", bufs=3)
                    nc.scalar.copy(ev[:], ps[:])
                    nc.sync.dma_start(
                        win[:, sub * (QS // 2) + k0:
                            sub * (QS // 2) + k0 + kn],
                        ev[:])

        def load_sums(si, name):
            o = smalls.tile([128, W], F32, name=name, tag="nw", bufs=16)
            src = sums_all.ap()[:, si * Nshp:(si + 1) * Nshp].rearrange(
                "a (p w) -> (a p) w", p=128)
            nc.sync.dma_start(o[:], src)
            return o

        # ------------- layer 1 -------------
        xgb = datp.tile([128, GSP], BF16, tag="base", name="xgb", bufs=1)
        _qs = (nc.sync, nc.scalar)
        for sg in range(NSIG):
            _qs[sg % 2].dma_start(
                xgb[4 * sg:4 * sg + 4, 0:GS],
                xb_full.ap()[:, sg * GS:(sg + 1) * GS].partition_broadcast(4))
        nc.vector.memset(xgb[:, GS:GSP], 0.0)

        psrc = sg_pass(xgb, "psrc1")
        pv = impulse_bcast(x_shardb, "pv")

        tmp = S("tmp1")
        nc.vector.tensor_scalar(out=tmp[:], in0=pv[:], scalar1=c2[:],
                                scalar2=None, op0=ALU.mult)
        epre = S("epre")
        nc.vector.scalar_tensor_tensor(out=epre[:], in0=psrc[:], scalar=c1[:],
                                       in1=tmp[:], op0=ALU.mult, op1=ALU.add)
        ae = S("ae")
        nc.vector.scalar_tensor_tensor(out=ae[:], in0=epre[:], scalar=0.2,
                                       in1=epre[:], op0=ALU.mult, op1=ALU.max)
        numer = S("numer")
        nc.scalar.activation(numer[:], ae[:], AF.Exp, bias=gneg1[:])
        w1s = S("w1s")
        nc.vector.tensor_tensor(out=w1s[:], in0=numer[:], in1=psrc[:],
                                op=ALU.mult)
        s0 = seg_scan(numer, "s0")
        s1 = seg_scan(w1s, "s1")
        bscatter(s0, 0, "s0")
        bscatter(s1, 1, "s1")

        den1 = load_sums(0, "den1")
        P1 = load_sums(1, "P1")
        # self-loop terms, affine in node layout
        xn = nodep.tile([128, W], F32, name="xn")
        nc.sync.dma_start(xn[:], x_shard.ap().rearrange(
            "a (p w) -> (a p) w", p=128))
        se1 = smalls.tile([128, W], F32, name="se1", tag="nw", bufs=16)
        nc.vector.tensor_scalar(out=se1[:], in0=xn[:], scalar1=c1c2[:],
                                scalar2=None, op0=ALU.mult)
        sl1 = smalls.tile([128, W], F32, name="sl1", tag="nw", bufs=16)
        nc.vector.scalar_tensor_tensor(out=sl1[:], in0=se1[:], scalar=0.2,
                                       in1=se1[:], op0=ALU.mult, op1=ALU.max)
        selfn1 = smalls.tile([128, W], F32, name="selfn1", tag="nw", bufs=16)
        nc.scalar.activation(selfn1[:], sl1[:], AF.Exp, bias=gneg1[:])
        nc.vector.tensor_tensor(out=den1[:], in0=den1[:], in1=selfn1[:],
                                op=ALU.add)
        sxp = smalls.tile([128, W], F32, name="sxp", tag="nw", bufs=16)
        nc.vector.tensor_tensor(out=sxp[:], in0=selfn1[:], in1=xn[:],
                                op=ALU.mult)
        nc.vector.tensor_tensor(out=P1[:], in0=P1[:], in1=sxp[:],
                                op=ALU.add)
        den1e = smalls.tile([128, W], F32, name="den1e", tag="nw", bufs=16)
        nc.vector.tensor_scalar(out=den1e[:], in0=den1[:], scalar1=1e-30,
                                scalar2=None, op0=ALU.add)
        rec1 = smalls.tile([128, W], F32, name="rec1", tag="nw", bufs=16)
        nc.vector.reciprocal(rec1[:], den1e[:])
        Pn = nodep.tile([128, W], F32, name="Pn")
        nc.vector.tensor_tensor(out=Pn[:], in0=P1[:], in1=rec1[:],
                                op=ALU.mult)
        # zero dummy-dst tail
        if Nsh < 128 * W:
            zt = smalls.tile([1, W], F32, name="zt")
            nc.vector.memset(zt[:], 0.0)
            for pz in range(Nsh // W, 128):
                a = max(0, Nsh - pz * W)
                if a < W:
                    nc.sync.dma_start(Pn[pz:pz + 1, a:W], zt[0:1, a:W])

        Pnb = nodep.tile([128, W], BF16, name="Pnb")
        nc.scalar.copy(Pnb[:], Pn[:])
        nc.sync.dma_start(p_localb.ap(), Pnb[:])

        # ------------- layer 2 node arrays (pre-collective) -------------
        rpn = nodep.tile([128, W], F32, name="rpn")
        nc.scalar.activation(rpn[:], Pn[:], AF.Relu)
        v2a = smalls.tile([128, W], F32, name="v2a", tag="nw", bufs=16)
        nc.vector.tensor_scalar(out=v2a[:], in0=rpn[:], scalar1=C2D2[:],
                                scalar2=None, op0=ALU.mult)
        v2sh = nodep.tile([128, W], F32, name="v2sh")
        nc.vector.scalar_tensor_tensor(out=v2sh[:], in0=Pn[:], scalar=nD2[:],
                                       in1=v2a[:], op0=ALU.mult, op1=ALU.add)
        v2shb = nodep.tile([128, W], BF16, name="v2shb")
        nc.scalar.copy(v2shb[:], v2sh[:])
        nc.sync.dma_start(v2_local.ap(), v2shb[:])
        pv2 = impulse_bcast(v2_local, "pv2")

        if no_collective:
            for cc_ in range(8):
                nc.sync.dma_start(p_fullb.ap()[:, cc_ * Nshp:(cc_ + 1) * Nshp],
                                  p_localb.ap())
        else:
            nc.gpsimd.collective_compute(
                "AllGather", ALU.bypass, replica_groups=[list(range(8))],
                ins=[p_localb.ap()], outs=[p_fullb.ap()])

        big2 = nodep.tile([128, PF], BF16, tag="gmaxb", name="pf_big", bufs=1)
        nc.sync.dma_start(big2[:], p_fullb.ap())
        mpp_i = nodep.tile([128, PF], F32, tag="gmaxr", name="pf_rp", bufs=1)
        nc.scalar.activation(mpp_i[:], big2[:], AF.Relu)
        mpp = cross_max(mpp_i, "mpp")
        mpm_i = nodep.tile([128, PF], F32, tag="gmaxr", name="pf_rm", bufs=1)
        nc.scalar.activation(mpm_i[:], big2[:], AF.Relu, scale=-1.0)
        mpm = cross_max(mpm_i, "mpm")

        def ub_pos(ca, cb, name):
            t1 = tmul(ca, mpp, name + "_1")
            r1 = smalls.tile([128, 1], F32, name=name + "_r1")
            nc.scalar.activation(r1[:], t1[:], AF.Relu)
            t2 = tmul(cb, mpm, name + "_2")
            r2 = smalls.tile([128, 1], F32, name=name + "_r2")
            nc.scalar.activation(r2[:], t2[:], AF.Relu)
            o = smalls.tile([128, 1], F32, name=name)
            nc.vector.tensor_tensor(out=o[:], in0=r1[:], in1=r2[:], op=ALU.add)
            return o

        ub2 = smalls.tile([128, 1], F32, name="ub2")
        nc.vector.tensor_tensor(out=ub2[:], in0=ub_pos(A2, B2, "ubu2")[:],
                                in1=ub_pos(C2, D2, "ubv2")[:], op=ALU.add)
        gneg2 = lrelu_neg(ub2, "gneg2")

        # ------------- layer 2 edges -------------
        pgb = datp.tile([128, GSP], BF16, tag="base", name="pgb", bufs=1)
        for sg in range(NSIG):
            coff = (sg // 4) * Nshp + (sg % 4) * GS
            _qs[sg % 2].dma_start(
                pgb[4 * sg:4 * sg + 4, 0:GS],
                p_fullb.ap()[:, coff:coff + GS].partition_broadcast(4))
        nc.vector.memset(pgb[:, GS:GSP], 0.0)
        psrc2 = sg_pass(pgb, "psrc2")

        rp = S("rp")
        nc.scalar.activation(rp[:], psrc2[:], AF.Relu)
        tmp2 = S("tmp2")
        nc.vector.tensor_scalar(out=tmp2[:], in0=psrc2[:], scalar1=nB2[:],
                                scalar2=None, op0=ALU.mult)
        u2 = S("u2")
        nc.vector.scalar_tensor_tensor(out=u2[:], in0=rp[:], scalar=A2B2[:],
                                       in1=tmp2[:], op0=ALU.mult, op1=ALU.add)
        epre2 = S("epre2")
        nc.vector.tensor_tensor(out=epre2[:], in0=u2[:], in1=pv2[:],
                                op=ALU.add)
        ae2 = S("ae2")
        nc.vector.scalar_tensor_tensor(out=ae2[:], in0=epre2[:], scalar=0.2,
                                       in1=epre2[:], op0=ALU.mult,
                                       op1=ALU.max)
        numer2 = S("numer2")
        nc.scalar.activation(numer2[:], ae2[:], AF.Exp, bias=gneg2[:])
        w21 = S("w21")
        nc.vector.tensor_tensor(out=w21[:], in0=numer2[:], in1=rp[:],
                                op=ALU.mult)
        w1b = S("w1b")
        nc.vector.tensor_tensor(out=w1b[:], in0=numer2[:], in1=psrc2[:],
                                op=ALU.mult)
        t0 = seg_scan(numer2, "t0")
        t1 = seg_scan(w21, "t1")
        t2 = seg_scan(w1b, "t2")
        bscatter(t0, 2, "t0")
        bscatter(t1, 3, "t1")
        bscatter(t2, 4, "t2")

        den2 = load_sums(2, "den2")
        Sp = load_sums(3, "Sp")
        Sraw = load_sums(4, "Sraw")
        # layer-2 self terms
        u2n = smalls.tile([128, W], F32, name="u2n", tag="nw", bufs=16)
        nc.vector.tensor_scalar(out=u2n[:], in0=rpn[:], scalar1=A2B2[:],
                                scalar2=None, op0=ALU.mult)
        u2n2 = smalls.tile([128, W], F32, name="u2n2", tag="nw", bufs=16)
        nc.vector.scalar_tensor_tensor(out=u2n2[:], in0=Pn[:], scalar=nB2[:],
                                       in1=u2n[:], op0=ALU.mult, op1=ALU.add)
        e2n = smalls.tile([128, W], F32, name="e2n", tag="nw", bufs=16)
        nc.vector.tensor_tensor(out=e2n[:], in0=u2n2[:], in1=v2sh[:],
                                op=ALU.add)
        sl2 = smalls.tile([128, W], F32, name="sl2", tag="nw", bufs=16)
        nc.vector.scalar_tensor_tensor(out=sl2[:], in0=e2n[:], scalar=0.2,
                                       in1=e2n[:], op0=ALU.mult, op1=ALU.max)
        selfn2 = smalls.tile([128, W], F32, name="selfn2", tag="nw", bufs=16)
        nc.scalar.activation(selfn2[:], sl2[:], AF.Exp, bias=gneg2[:])
        nc.vector.tensor_tensor(out=den2[:], in0=den2[:], in1=selfn2[:],
                                op=ALU.add)
        srp = smalls.tile([128, W], F32, name="srp", tag="nw", bufs=16)
        nc.vector.tensor_tensor(out=srp[:], in0=selfn2[:], in1=rpn[:],
                                op=ALU.mult)
        nc.vector.tensor_tensor(out=Sp[:], in0=Sp[:], in1=srp[:],
                                op=ALU.add)
        srw = smalls.tile([128, W], F32, name="srw", tag="nw", bufs=16)
        nc.vector.tensor_tensor(out=srw[:], in0=selfn2[:], in1=Pn[:],
                                op=ALU.mult)
        nc.vector.tensor_tensor(out=Sraw[:], in0=Sraw[:], in1=srw[:],
                                op=ALU.add)
        den2e = smalls.tile([128, W], F32, name="den2e", tag="nw", bufs=16)
        nc.vector.tensor_scalar(out=den2e[:], in0=den2[:], scalar1=1e-30,
                                scalar2=None, op0=ALU.add)
        rec2 = smalls.tile([128, W], F32, name="rec2", tag="nw", bufs=16)
        nc.vector.reciprocal(rec2[:], den2e[:])
        Rp2 = smalls.tile([128, W], F32, name="Rp2", tag="nw", bufs=16)
        nc.vector.tensor_tensor(out=Rp2[:], in0=Sp[:], in1=rec2[:],
                                op=ALU.mult)
        Smm = smalls.tile([128, W], F32, name="Smm", tag="nw", bufs=16)
        nc.vector.tensor_tensor(out=Smm[:], in0=Sp[:], in1=Sraw[:],
                                op=ALU.subtract)
        Rm = smalls.tile([128, W], F32, name="Rm", tag="nw", bufs=16)
        nc.vector.tensor_tensor(out=Rm[:], in0=Smm[:], in1=rec2[:],
                                op=ALU.mult)

        # y[d] = bl + sum_k relu(Rp*qp_k + Rm*qm_k + b2_k) * Wl_k
        yk = smalls.tile([128, W * 20], F32, name="yk", tag="yka", bufs=1)
        yk3 = yk[:].rearrange("p (w k) -> p w k", k=20)
        nc.vector.tensor_tensor(
            out=yk3,
            in0=Rp2[:].unsqueeze(2).broadcast_to([128, W, 20]),
            in1=qp[:].unsqueeze(1).broadcast_to([128, W, 20]), op=ALU.mult)
        yk2 = smalls.tile([128, W * 20], F32, name="yk2", tag="ykb", bufs=1)
        yk23 = yk2[:].rearrange("p (w k) -> p w k", k=20)
        nc.vector.tensor_tensor(
            out=yk23,
            in0=Rm[:].unsqueeze(2).broadcast_to([128, W, 20]),
            in1=qm[:].unsqueeze(1).broadcast_to([128, W, 20]), op=ALU.mult)
        nc.vector.tensor_tensor(out=yk[:], in0=yk[:], in1=yk2[:], op=ALU.add)
        nc.vector.tensor_tensor(
            out=yk3, in0=yk3,
            in1=b2t[:].unsqueeze(1).broadcast_to([128, W, 20]), op=ALU.add)
        nc.scalar.activation(yk[:], yk[:], AF.Relu)
        nc.vector.tensor_tensor(
            out=yk3, in0=yk3,
            in1=wlt[:].unsqueeze(1).broadcast_to([128, W, 20]), op=ALU.mult)
        yacc = smalls.tile([128, W], F32, name="yacc", tag="nw", bufs=16)
        nc.vector.tensor_reduce(yacc[:], yk3, AX.X, ALU.add)
        yf = smalls.tile([128, W], F32, name="yf", tag="nw", bufs=16)
        nc.vector.tensor_scalar(out=yf[:], in0=yacc[:], scalar1=blt[:],
                                scalar2=None, op0=ALU.add)
        nc.sync.dma_start(y_out.ap(), yf[:])

    nc.compile()
    return nc


def make_in_maps(pp, inputs):
    N, Nsh, Nshp = pp["N"], pp["Nsh"], pp["Nshp"]
    NF = -(-N // 128)
    x = np.asarray(inputs["x"], np.float32).reshape(-1)
    x_full = np.zeros(128 * NF, np.float32)
    x_full[:N] = x
    xb_full = x.astype(ml_dtypes.bfloat16)
    W2T = np.ascontiguousarray(np.asarray(inputs["W2"], np.float32).T)

    common = {
        "x_full": x_full[None, :],
        "xb_full": xb_full[None, :],
        "W1": np.asarray(inputs["W1"], np.float32).reshape(1, 20),
        "a_src1": np.asarray(inputs["a_src1"], np.float32).reshape(1, 20),
        "a_dst1": np.asarray(inputs["a_dst1"], np.float32).reshape(1, 20),
        "W2T": W2T.reshape(1, 400),
        "a_src2": np.asarray(inputs["a_src2"], np.float32).reshape(1, 20),
        "a_dst2": np.asarray(inputs["a_dst2"], np.float32).reshape(1, 20),
        "b2": np.asarray(inputs["b2"], np.float32).reshape(1, 20),
        "Wl": np.asarray(inputs["Wl"], np.float32).reshape(1, 20),
        "bl": np.asarray(inputs["bl"], np.float32).reshape(1, 1),
        "w4": pp["w4"],
    }
    maps = []
    for c in range(8):
        pc = pp["cores"][c]
        xs = np.zeros(Nshp, np.float32)
        xs[:Nsh] = x[c * Nsh:(c + 1) * Nsh]
        maps.append({
            **common,
            "x_shard": xs[None, :],
            "x_shardb": xs[None, :].astype(ml_dtypes.bfloat16),
            "runstart": pc["runstart"].astype(ml_dtypes.bfloat16),
            "impA": pc["impA"],
            "impB": pc["impB"],
            "bidx0": pc["bidx0"],
            "bidx1": pc["bidx1"],
            "sgA": pc["sgA"],
            "sgB": pc["sgB"],
        })
    return maps


def kernel(**inputs):
    x = np.asarray(inputs["x"], np.float32)
    N = x.shape[0]
    # device path assumes b1 == 0 (true for this problem) plus the layout
    # asserts in prep; fall back to numpy on anything unexpected.
    if np.any(np.asarray(inputs["b1"])) or N != N_NODES:
        return _kernel_numpy(**inputs)
    try:
        pp = prep(np.asarray(inputs["edge_index"]), N)
        nc = build(pp)
        maps = make_in_maps(pp, inputs)
    except Exception:
        return _kernel_numpy(**inputs)
    from concourse.bass_utils import run_bass_kernel_spmd
    res = run_bass_kernel_spmd(nc, maps, list(range(8)))
    Nsh = pp["Nsh"]
    y = np.zeros((N, 1), np.float32)
    for c in range(8):
        y[c * Nsh:(c + 1) * Nsh, 0] = res.results[c]["y"].reshape(-1)[:Nsh]
    return y


def _kernel_numpy(x, edge_index, W1, a_src1, a_dst1, b1, W2, a_src2, a_dst2,
                  b2, Wl, bl):
    def lr(v):
        return np.where(v > 0, v, 0.2 * v).astype(np.float32)

    def conv(h, src, dst, Wm, asrc, adst, b, n):
        hh = (h @ Wm).astype(np.float32)
        u, v = hh @ asrc, hh @ adst
        e = lr(u[src] + v[dst])
        m = np.full(n, -np.inf, np.float32)
        np.maximum.at(m, dst, e)
        ee = np.exp(e - m[dst]).astype(np.float32)
        den = np.bincount(dst, weights=ee, minlength=n).astype(np.float32)
        al = ee / (den[dst] + 1e-16)
        out = np.zeros((n, hh.shape[1]), np.float32)
        wh = hh[src] * al[:, None]
        for k in range(hh.shape[1]):
            out[:, k] = np.bincount(dst, weights=wh[:, k], minlength=n)
        return out + b

    n = x.shape[0]
    loop = np.arange(n, dtype=np.int64)
    src = np.concatenate([edge_index[0], loop])
    dst = np.concatenate([edge_index[1], loop])
    h = np.maximum(conv(np.asarray(x, np.float32), src, dst, W1, a_src1,
                        a_dst1, b1, n), 0)
    h = np.maximum(conv(h, src, dst, W2, a_src2, a_dst2, b2, n), 0)
    return (h @ Wl + bl).astype(np.float32)


# revision 19
# speedup vs baseline: 1.1501x; 1.1501x over previous
"""Trainium2 Bass kernel v4: scatter-gather edges via gpsimd local_scatter.

Layout: 8 cores shard by dst (12500 nodes each). Per core, 128 SBUF
partitions = (src-group sigma in [0,32) global, 3125 nodes each) x
(dst-quarter h in [0,4) of the core's shard, 3136-padded). Partition
p = sigma*4 + h holds its edges sorted by dst-local.

Per-edge src values (psrc): local_scatter with data = the group's node
values repeated R times (R = max edge multiplicity per (partition, src));
idx[(r, j)] = stream slot of the r-th edge of src j, -1 if none. Replaces
the ap_gather passes (~26ns/idx) with ~2ns/idx scatters.

Per-edge dst values (pv): impulse local_scatter at run starts + masked
scan broadcast (as before). Per-dst segment sums: local_scatter run-end
scan values to dst slots + one-hot PE matmul collapse over sigma.
"""
import numpy as np
import ml_dtypes

N_NODES = 100000

# ===================== tile drain workaround =====================
import concourse.tile as tile
from concourse import mybir
from bass_rust import ScopedClock


def _patched_drain_and_barrier(self, tick_clock, wait_clock):
    nc = self.nc
    probe = nc.sync.nop()
    wait_clock.add_sem_waits(probe.ins, ScopedClock({None: tick_clock.global_clock}))
    si = probe.ins.sync_info
    waits = list(si.on_wait) if si is not None else []
    if si is not None:
        si.on_wait = waits[:1]
    for w in waits[1:]:
        nop = nc.sync.nop()
        nop.ins.sync_info = mybir.SyncInfo(on_wait=[w], on_update=[])
    nc.sync.drain()
    nc.all_engine_barrier()
    popped = nc._tile_sem_poison_stack.pop()
    assert popped is self._sem_poison
    nc.clear_and_free_semaphores(list(self.sems.allocated().values()))
    nc.all_engine_barrier()


def install():
    tile.TileContext._drain_and_barrier = _patched_drain_and_barrier


install()

from contextlib import ExitStack
import concourse.bass as bass
import concourse.bacc as bacc

F32 = mybir.dt.float32
I16 = mybir.dt.int16
BF16 = mybir.dt.bfloat16
AF = mybir.ActivationFunctionType
ALU = mybir.AluOpType
AX = mybir.AxisListType

NSIG = 32          # global src groups
GS = 3125          # nodes per group
GSP = 3126         # padded (even) data cols per group
NH = 4             # dst quarters per core
QS = 3136          # padded quarter size (12544/4)
NCH = 5            # round chunks per half


def prep(edge_index: np.ndarray, N: int):
    n_cores = 8
    assert N == N_NODES and N % n_cores == 0
    Nsh = N // n_cores            # 12500
    Nshp = 12544
    W = Nshp // 128               # 98

    src = edge_index[0].astype(np.int64)
    dst = edge_index[1].astype(np.int64)
    core = dst // Nsh
    dstl = dst - core * Nsh
    sig = src // GS
    h = dstl // QS
    p = sig * 4 + h
    key = (core * 128 + p) * Nsh + dstl
    order = np.argsort(key, kind="stable")
    src_s = src[order]
    dstl_s = dstl[order].astype(np.int32)
    cp = (core * 128 + p)[order]

    cp_starts = np.searchsorted(cp, np.arange(n_cores * 128))
    cp_ends = np.searchsorted(cp, np.arange(n_cores * 128), side="right")
    counts = cp_ends - cp_starts
    L = int(counts.max())
    L = -(-L // 16) * 16
    LH = L // 2
    assert LH <= 2046 and LH % 2 == 0, f"LH={LH}"

    def occ_of(j):
        """occurrence index among equal values, preserving order"""
        n = len(j)
        if n == 0:
            return np.zeros(0, dtype=np.int64)
        o2 = np.argsort(j, kind="stable")
        js = j[o2]
        newg = np.empty(n, dtype=bool)
        newg[0] = True
        np.not_equal(js[1:], js[:-1], out=newg[1:])
        start_pos = np.maximum.accumulate(np.where(newg, np.arange(n), 0))
        occ = np.empty(n, dtype=np.int64)
        occ[o2] = np.arange(n) - start_pos
        return occ

    # first pass: max multiplicity R, counted PER STREAM-HALF (each half
    # is its own scatter call, so only within-half uniqueness matters)
    R = 1
    occs = []
    for ci in range(n_cores * 128):
        a, b = cp_starts[ci], cp_ends[ci]
        j = (src_s[a:b] % GS).astype(np.int64)
        n = b - a
        mA = np.arange(n) < LH
        occA = occ_of(j[mA])
        occB = occ_of(j[~mA])
        if len(occA):
            R = max(R, int(occA.max()) + 1)
        if len(occB):
            R = max(R, int(occB.max()) + 1)
        occs.append((occA, occB))
    RH = 2
    Rp = -(-R // RH) * RH
    SGW = Rp * GSP

    cores = []
    for c in range(n_cores):
        runstart = np.ones((128, L), dtype=np.float32)
        impA = np.full((128, QS), -1, dtype=np.int16)
        impB = np.full((128, QS), -1, dtype=np.int16)
        bidx0 = np.full((128, L), -1, dtype=np.int16)
        bidx1 = np.full((128, L), -1, dtype=np.int16)
        sgA = np.full((128, SGW), -1, dtype=np.int16)
        sgB = np.full((128, SGW), -1, dtype=np.int16)

        for pi in range(128):
            ci = c * 128 + pi
            a, b = cp_starts[ci], cp_ends[ci]
            n = b - a
            if n == 0:
                continue
            hq = pi % 4
            dd = dstl_s[a:b]
            jj = (src_s[a:b] % GS).astype(np.int64)
            occA, occB = occs[ci]
            slots = np.arange(n)
            inA = slots < LH
            posA = occA * GSP + jj[inA]
            posB = occB * GSP + jj[~inA]
            sgA[pi][posA] = slots[inA].astype(np.int16)
            sgB[pi][posB] = (slots[~inA] - LH).astype(np.int16)

            newrun = np.empty(n, dtype=bool)
            newrun[0] = True
            np.not_equal(dd[1:], dd[:-1], out=newrun[1:])
            rs = np.flatnonzero(newrun)
            runstart[pi, rs] = 0.0
            run_dst = dd[rs] - hq * QS          # dst pos within quarter
            run_ends = np.append(rs[1:], n) - 1
            lo = rs < LH
            impA[pi, run_dst[lo]] = rs[lo].astype(np.int16)
            impB[pi, run_dst[~lo]] = (rs[~lo] - LH).astype(np.int16)
            sub0 = run_dst < (QS // 2)
            bidx0[pi, run_ends[sub0]] = run_dst[sub0].astype(np.int16)
            bidx1[pi, run_ends[~sub0]] = (run_dst[~sub0] -
                                          QS // 2).astype(np.int16)

        cores.append(dict(runstart=runstart, impA=impA, impB=impB,
                          bidx0=bidx0, bidx1=bidx1, sgA=sgA, sgB=sgB))

    w4 = np.zeros((128, 4), dtype=np.float32)
    w4[np.arange(128), np.arange(128) % 4] = 1.0

    return dict(N=N, Nsh=Nsh, Nshp=Nshp, W=W, L=L, LH=LH, R=R, RH=RH,
                Rp=Rp, SGW=SGW, cores=cores, w4=w4)


def build(pp, no_collective=False):
    N, Nsh, Nshp, W = pp["N"], pp["Nsh"], pp["Nshp"], pp["W"]
    L, LH, RH, Rp, SGW = pp["L"], pp["LH"], pp["RH"], pp["Rp"], pp["SGW"]
    R = pp["R"]
    NF = -(-N // 128)
    PF = (8 * Nshp) // 128
    CW = RH * GSP                   # idx cols per round-chunk call

    nc = bacc.Bacc("TRN2", target_bir_lowering=False, debug=False,
                   num_devices=8)

    def din(name, shape, dt=F32):
        return nc.dram_tensor(name, shape, dt, kind="ExternalInput")

    x_full = din("x_full", [1, 128 * NF])
    xb_full = din("xb_full", [1, NSIG * GS], BF16)
    x_shard = din("x_shard", [1, Nshp])
    x_shardb = din("x_shardb", [1, Nshp], BF16)
    W1 = din("W1", [1, 20])
    a_src1 = din("a_src1", [1, 20])
    a_dst1 = din("a_dst1", [1, 20])
    W2T = din("W2T", [1, 400])
    a_src2 = din("a_src2", [1, 20])
    a_dst2 = din("a_dst2", [1, 20])
    b2 = din("b2", [1, 20])
    Wl = din("Wl", [1, 20])
    bl = din("bl", [1, 1])
    runst = din("runstart", [128, L], BF16)
    impAd = din("impA", [128, QS], I16)
    impBd = din("impB", [128, QS], I16)
    bidx0d = din("bidx0", [128, L], I16)
    bidx1d = din("bidx1", [128, L], I16)
    sgAd = din("sgA", [128, SGW], I16)
    sgBd = din("sgB", [128, SGW], I16)
    w4d = din("w4", [128, 4])

    y_out = nc.dram_tensor("y", [1, Nshp], F32, kind="ExternalOutput")

    p_localb = nc.dram_tensor("p_localb", [1, Nshp], BF16)
    p_fullb = nc.dram_tensor("p_fullb", [1, 8 * Nshp], BF16,
                             addr_space="Shared")
    v2_local = nc.dram_tensor("v2_local", [1, Nshp], BF16)
    rt128 = nc.dram_tensor("rt128", [1, 128], F32)
    rt1 = nc.dram_tensor("rt1", [1, 1], F32)
    sums_all = nc.dram_tensor("sums_all", [1, 5 * Nshp], F32)

    with tile.TileContext(nc) as tc, ExitStack() as ctx:
        consts = ctx.enter_context(tc.tile_pool(name="consts", bufs=1))
        smalls = ctx.enter_context(tc.tile_pool(name="smalls", bufs=2))
        strm = ctx.enter_context(tc.tile_pool(name="strm", bufs=5))
        sgp = ctx.enter_context(tc.tile_pool(name="sg", bufs=2))
        datp = ctx.enter_context(tc.tile_pool(name="dat", bufs=1))
        halfp = ctx.enter_context(tc.tile_pool(name="half", bufs=4))
        impp = ctx.enter_context(tc.tile_pool(name="imp", bufs=1))
        bnd = ctx.enter_context(tc.tile_pool(name="bnd", bufs=3))
        nodep = ctx.enter_context(tc.tile_pool(name="node", bufs=1))
        psp = ctx.enter_context(tc.tile_pool(name="ps", bufs=2, space="PSUM"))

        def S(name, dt=BF16):
            return strm.tile([128, L], dt, tag="s", name=name, bufs=5)

        def bcast(dram_ap, n, name):
            t = consts.tile([128, n], F32, name=name)
            nc.sync.dma_start(t[:], dram_ap.partition_broadcast(128))
            return t

        def rsc(t, name, op=ALU.add):
            out = consts.tile([128, 1], F32, name=name)
            nc.vector.tensor_reduce(out[:], t[:], AX.X, op)
            return out

        def tmul(a, b, name, pool=None):
            out = (pool or smalls).tile(list(a.shape), F32, name=name)
            nc.vector.tensor_tensor(out=out[:], in0=a[:], in1=b[:],
                                    op=ALU.mult)
            return out

        # ------------- constants -------------
        wb = bcast(W1.ap(), 20, "wb")
        a1s = bcast(a_src1.ap(), 20, "a1s")
        a1d = bcast(a_dst1.ap(), 20, "a1d")
        a2s = bcast(a_src2.ap(), 20, "a2s")
        a2d = bcast(a_dst2.ap(), 20, "a2d")
        b2t = bcast(b2.ap(), 20, "b2t")
        wlt = bcast(Wl.ap(), 20, "wlt")
        blt = bcast(bl.ap(), 1, "blt")
        w2t = bcast(W2T.ap(), 400, "w2t")

        c1 = rsc(tmul(wb, a1s, "c1m"), "c1")
        c2 = rsc(tmul(wb, a1d, "c2m"), "c2")
        c1c2 = consts.tile([128, 1], F32, name="c1c2")
        nc.vector.tensor_tensor(out=c1c2[:], in0=c1[:], in1=c2[:], op=ALU.add)

        wp = consts.tile([128, 20], F32, name="wp")
        nc.scalar.activation(wp[:], wb[:], AF.Relu)
        wm = consts.tile([128, 20], F32, name="wm")
        nc.scalar.activation(wm[:], wb[:], AF.Relu, scale=-1.0)

        def qvec(wv, name):
            t = smalls.tile([128, 400], F32, name=name + "_t", tag="q400",
                            bufs=1)
            nc.vector.tensor_tensor(
                out=t[:], in0=w2t[:],
                in1=wv[:].unsqueeze(1).broadcast_to([128, 20, 20]),
                op=ALU.mult)
            out = consts.tile([128, 20], F32, name=name)
            nc.vector.tensor_reduce(
                out[:], t[:].rearrange("p (j k) -> p j k", j=20), AX.X,
                ALU.add)
            return out

        qp = qvec(wp, "qp")
        qm = qvec(wm, "qm")
        A2 = rsc(tmul(qp, a2s, "A2m"), "A2")
        B2 = rsc(tmul(qm, a2s, "B2m"), "B2")
        C2 = rsc(tmul(qp, a2d, "C2m"), "C2")
        D2 = rsc(tmul(qm, a2d, "D2m"), "D2")
        A2B2 = consts.tile([128, 1], F32, name="A2B2")
        nc.vector.tensor_tensor(out=A2B2[:], in0=A2[:], in1=B2[:], op=ALU.add)
        nB2 = consts.tile([128, 1], F32, name="nB2")
        nc.scalar.mul(nB2[:], B2[:], -1.0)
        C2D2 = consts.tile([128, 1], F32, name="C2D2")
        nc.vector.tensor_tensor(out=C2D2[:], in0=C2[:], in1=D2[:], op=ALU.add)
        nD2 = consts.tile([128, 1], F32, name="nD2")
        nc.scalar.mul(nD2[:], D2[:], -1.0)

        w4b = consts.tile([128, 4], BF16, name="w4b")
        w4s = consts.tile([128, 4], F32, name="w4s")
        nc.sync.dma_start(w4s[:], w4d.ap())
        nc.scalar.copy(w4b[:], w4s[:])

        def cross_max(tin, name):
            m = smalls.tile([128, 1], F32, name=name + "_m")
            nc.vector.tensor_reduce(m[:], tin[:], AX.X, ALU.max)
            nc.sync.dma_start(rt128.ap(), m[:])
            row = smalls.tile([1, 128], F32, name=name + "_row")
            nc.sync.dma_start(row[:], rt128.ap())
            m1 = smalls.tile([1, 1], F32, name=name + "_m1")
            nc.vector.tensor_reduce(m1[:], row[:], AX.X, ALU.max)
            nc.sync.dma_start(rt1.ap(), m1[:])
            mb = consts.tile([128, 1], F32, name=name)
            nc.sync.dma_start(mb[:], rt1.ap().partition_broadcast(128))
            return mb

        def relu_pair_max(dram_ap, ncols, name, dt=F32):
            big = nodep.tile([128, ncols], dt, tag="gmax", name=name + "_big",
                             bufs=1)
            nc.sync.dma_start(big[:], dram_ap)
            outs = []
            for i, sgn in enumerate((1.0, -1.0)):
                r = nodep.tile([128, ncols], F32, tag="gmaxr",
                               name=f"{name}_r{i}", bufs=1)
                nc.scalar.activation(r[:], big[:], AF.Relu, scale=sgn)
                outs.append(cross_max(r, f"{name}{i}"))
            return outs

        def sc1(name):
            return consts.tile([128, 1], F32, name=name)

        def lrelu_neg(t, name):
            o = sc1(name + "_lr")
            nc.vector.scalar_tensor_tensor(out=o[:], in0=t[:], scalar=0.2,
                                           in1=t[:], op0=ALU.mult,
                                           op1=ALU.max)
            o2 = sc1(name)
            nc.scalar.mul(o2[:], o[:], -1.0)
            return o2

        mxp, mxm = relu_pair_max(x_full.ap(), NF, "mx")

        def ub_exact(cc, name):
            t1 = tmul(cc, mxp, name + "_1")
            ncc = smalls.tile([128, 1], F32, name=name + "_n")
            nc.scalar.mul(ncc[:], cc[:], -1.0)
            t2 = tmul(ncc, mxm, name + "_2")
            o = smalls.tile([128, 1], F32, name=name)
            nc.vector.tensor_tensor(out=o[:], in0=t1[:], in1=t2[:], op=ALU.max)
            return o

        ub1 = smalls.tile([128, 1], F32, name="ub1")
        nc.vector.tensor_tensor(out=ub1[:], in0=ub_exact(c1, "ubu1")[:],
                                in1=ub_exact(c2, "ubv1")[:], op=ALU.add)
        gneg1 = lrelu_neg(ub1, "gneg1")

        # ------------- shared tiles -------------
        runstart_t = nodep.tile([128, L], BF16, name="runstart_t")
        nc.sync.dma_start(runstart_t[:], runst.ap())
        bidx0_t = nodep.tile([128, L], I16, name="bidx0_t")
        nc.sync.dma_start(bidx0_t[:], bidx0d.ap())
        bidx1_t = nodep.tile([128, L], I16, name="bidx1_t")
        nc.sync.dma_start(bidx1_t[:], bidx1d.ap())
        impA_t = nodep.tile([128, QS], I16, name="impA_t")
        nc.sync.dma_start(impA_t[:], impAd.ap())
        impB_t = nodep.tile([128, QS], I16, name="impB_t")
        nc.sync.dma_start(impB_t[:], impBd.ap())

        def sg_pass(base, name):
            """psrc stream via repeated-table local_scatter rounds.
            base: [128, GSP] bf16 group-table tile.
            """
            rep = datp.tile([128, CW], BF16, tag="rep", name=name + "_rep",
                            bufs=1)
            for r in range(RH):
                nc.scalar.copy(rep[:, r * GSP:(r + 1) * GSP], base[:])
            psrc = strm.tile([128, L], BF16, tag="pk", name=name, bufs=1)
            nch_eff = -(-R // RH)
            for off, sgd in ((0, sgAd), (LH, sgBd)):
                acc = None
                for ch in range(nch_eff):
                    # trim the all-padding rounds in the last chunk
                    cw = min(CW, (R - ch * RH) * GSP)
                    it = sgp.tile([128, CW], I16, tag="sgi",
                                  name=f"{name}_i{off}_{ch}", bufs=2)
                    nc.sync.dma_start(
                        it[:, 0:cw], sgd.ap()[:, ch * CW:ch * CW + cw])
                    o = halfp.tile([128, LH], BF16, tag="ho",
                                   name=f"{name}_o{off}_{ch}", bufs=2)
                    nc.gpsimd.local_scatter(o[:], rep[:], it[:, 0:cw],
                                            channels=128, num_elems=LH,
                                            num_idxs=cw)
                    last = ch == nch_eff - 1
                    tgt = psrc[:, off:off + LH]
                    if acc is None:
                        if last:
                            nc.scalar.copy(tgt, o[:])
                        else:
                            acc = halfp.tile([128, LH], BF16, tag="hacc",
                                             name=f"{name}_a{off}", bufs=2)
                            nc.scalar.copy(acc[:], o[:])
                    else:
                        nc.vector.tensor_tensor(
                            out=tgt if last else acc[:], in0=acc[:],
                            in1=o[:], op=ALU.add)
            return psrc

        def impulse_bcast(src_dram, name):
            """[1, Nshp] bf16 DRAM -> per-partition dst-quarter values
            broadcast over runs: bf16 [128, L] stream."""
            stageb = impp.tile([128, QS], BF16, tag="impb",
                               name=name + "_sb", bufs=2)
            src_ap = src_dram.ap().rearrange("a (h j) -> (a h) j", h=4)
            for sg in range(NSIG):
                (nc.scalar if sg % 2 else nc.sync).dma_start(
                    stageb[4 * sg:4 * sg + 4, :], src_ap)
            imp = strm.tile([128, L], BF16, tag="imp", name=name + "_imp",
                            bufs=1)
            nc.gpsimd.local_scatter(imp[:, 0:LH], stageb[:], impA_t[:],
                                    channels=128, num_elems=LH, num_idxs=QS)
            nc.gpsimd.local_scatter(imp[:, LH:L], stageb[:], impB_t[:],
                                    channels=128, num_elems=L - LH,
                                    num_idxs=QS)
            out = S(name)
            nc.vector.tensor_tensor_scan(
                out[:], runstart_t[:], imp[:], 0.0, ALU.mult, ALU.add)
            return out

        def seg_scan(data, name):
            s = S(name)
            nc.vector.tensor_tensor_scan(
                s[:], runstart_t[:], data[:], 0.0, ALU.mult, ALU.add)
            return s

        def bscatter(sct, si, name):
            """Run-end extraction into dst-quarter slots + sigma-collapse."""
            win = sums_all.ap()[:, si * Nshp:(si + 1) * Nshp].rearrange(
                "a (h j) -> (a h) j", h=4)
            for sub, bt in ((0, bidx0_t), (1, bidx1_t)):
                out = bnd.tile([128, QS // 2], BF16, tag="bs",
                               name=f"bs_{name}_{sub}", bufs=3)
                nc.gpsimd.local_scatter(out[:], sct[:], bt[:],
                                        channels=128, num_elems=QS // 2,
                                        num_idxs=L)
                for k0 in range(0, QS // 2, 512):
                    kn = min(512, QS // 2 - k0)
                    ps = psp.tile([4, kn], F32, tag="ps",
                                  name=f"ps_{name}_{sub}_{k0}")
                    nc.tensor.matmul(ps[:], w4b[:], out[:, k0:k0 + kn],
                                     start=True, stop=True)
                    ev = bnd.tile([4, kn], F32, tag="ev",
                                  name=f"ev_{name}_{sub}_{k0}", bufs=3)
                    nc.scalar.copy(ev[:], ps[:])
                    nc.sync.dma_start(
                        win[:, sub * (QS // 2) + k0:
                            sub * (QS // 2) + k0 + kn],
                        ev[:])

        def load_sums(si, name):
            o = smalls.tile([128, W], F32, name=name, tag="nw", bufs=16)
            src = sums_all.ap()[:, si * Nshp:(si + 1) * Nshp].rearrange(
                "a (p w) -> (a p) w", p=128)
            nc.sync.dma_start(o[:], src)
            return o

        # ------------- layer 1 -------------
        xgb = datp.tile([128, GSP], BF16, tag="base", name="xgb", bufs=1)
        _qs = (nc.sync, nc.scalar)
        for sg in range(NSIG):
            _qs[sg % 2].dma_start(
                xgb[4 * sg:4 * sg + 4, 0:GS],
                xb_full.ap()[:, sg * GS:(sg + 1) * GS].partition_broadcast(4))
        nc.vector.memset(xgb[:, GS:GSP], 0.0)

        psrc = sg_pass(xgb, "psrc1")
        pv = impulse_bcast(x_shardb, "pv")

        tmp = S("tmp1")
        nc.vector.tensor_scalar(out=tmp[:], in0=pv[:], scalar1=c2[:],
                                scalar2=None, op0=ALU.mult)
        epre = S("epre")
        nc.vector.scalar_tensor_tensor(out=epre[:], in0=psrc[:], scalar=c1[:],
                                       in1=tmp[:], op0=ALU.mult, op1=ALU.add)
        ae = S("ae")
        nc.vector.scalar_tensor_tensor(out=ae[:], in0=epre[:], scalar=0.2,
                                       in1=epre[:], op0=ALU.mult, op1=ALU.max)
        numer = S("numer")
        nc.scalar.activation(numer[:], ae[:], AF.Exp, bias=gneg1[:])
        w1s = S("w1s")
        nc.vector.tensor_tensor(out=w1s[:], in0=numer[:], in1=psrc[:],
                                op=ALU.mult)
        s0 = seg_scan(numer, "s0")
        s1 = seg_scan(w1s, "s1")
        bscatter(s0, 0, "s0")
        bscatter(s1, 1, "s1")

        den1 = load_sums(0, "den1")
        P1 = load_sums(1, "P1")
        # self-loop terms, affine in node layout
        xn = nodep.tile([128, W], F32, name="xn")
        nc.sync.dma_start(xn[:], x_shard.ap().rearrange(
            "a (p w) -> (a p) w", p=128))
        se1 = smalls.tile([128, W], F32, name="se1", tag="nw", bufs=16)
        nc.vector.tensor_scalar(out=se1[:], in0=xn[:], scalar1=c1c2[:],
                                scalar2=None, op0=ALU.mult)
        sl1 = smalls.tile([128, W], F32, name="sl1", tag="nw", bufs=16)
        nc.vector.scalar_tensor_tensor(out=sl1[:], in0=se1[:], scalar=0.2,
                                       in1=se1[:], op0=ALU.mult, op1=ALU.max)
        selfn1 = smalls.tile([128, W], F32, name="selfn1", tag="nw", bufs=16)
        nc.scalar.activation(selfn1[:], sl1[:], AF.Exp, bias=gneg1[:])
        nc.vector.tensor_tensor(out=den1[:], in0=den1[:], in1=selfn1[:],
                                op=ALU.add)
        sxp = smalls.tile([128, W], F32, name="sxp", tag="nw", bufs=16)
        nc.vector.tensor_tensor(out=sxp[:], in0=selfn1[:], in1=xn[:],
                                op=ALU.mult)
        nc.vector.tensor_tensor(out=P1[:], in0=P1[:], in1=sxp[:],
                                op=ALU.add)
        den1e = smalls.tile([128, W], F32, name="den1e", tag="nw", bufs=16)
        nc.vector.tensor_scalar(out=den1e[:], in0=den1[:], scalar1=1e-30,
                                scalar2=None, op0=ALU.add)
        rec1 = smalls.tile([128, W], F32, name="rec1", tag="nw", bufs=16)
        nc.vector.reciprocal(rec1[:], den1e[:])
        Pn = nodep.tile([128, W], F32, name="Pn")
        nc.vector.tensor_tensor(out=Pn[:], in0=P1[:], in1=rec1[:],
                                op=ALU.mult)
        # zero dummy-dst tail
        if Nsh < 128 * W:
            zt = smalls.tile([1, W], F32, name="zt")
            nc.vector.memset(zt[:], 0.0)
            for pz in range(Nsh // W, 128):
                a = max(0, Nsh - pz * W)
                if a < W:
                    nc.sync.dma_start(Pn[pz:pz + 1, a:W], zt[0:1, a:W])

        Pnb = nodep.tile([128, W], BF16, name="Pnb")
        nc.scalar.copy(Pnb[:], Pn[:])
        nc.sync.dma_start(p_localb.ap(), Pnb[:])

        # ------------- layer 2 node arrays (pre-collective) -------------
        rpn = nodep.tile([128, W], F32, name="rpn")
        nc.scalar.activation(rpn[:], Pn[:], AF.Relu)
        v2a = smalls.tile([128, W], F32, name="v2a", tag="nw", bufs=16)
        nc.vector.tensor_scalar(out=v2a[:], in0=rpn[:], scalar1=C2D2[:],
                                scalar2=None, op0=ALU.mult)
        v2sh = nodep.tile([128, W], F32, name="v2sh")
        nc.vector.scalar_tensor_tensor(out=v2sh[:], in0=Pn[:], scalar=nD2[:],
                                       in1=v2a[:], op0=ALU.mult, op1=ALU.add)
        v2shb = nodep.tile([128, W], BF16, name="v2shb")
        nc.scalar.copy(v2shb[:], v2sh[:])
        nc.sync.dma_start(v2_local.ap(), v2shb[:])
        pv2 = impulse_bcast(v2_local, "pv2")

        if no_collective:
            for cc_ in range(8):
                nc.sync.dma_start(p_fullb.ap()[:, cc_ * Nshp:(cc_ + 1) * Nshp],
                                  p_localb.ap())
        else:
            nc.gpsimd.collective_compute(
                "AllGather", ALU.bypass, replica_groups=[list(range(8))],
                ins=[p_localb.ap()], outs=[p_fullb.ap()])

        big2 = nodep.tile([128, PF], BF16, tag="gmaxb", name="pf_big", bufs=1)
        nc.scalar.dma_start(big2[:], p_fullb.ap())
        mpp_i = nodep.tile([128, PF], F32, tag="gmaxr", name="pf_rp", bufs=1)
        nc.scalar.activation(mpp_i[:], big2[:], AF.Relu)
        mpp = cross_max(mpp_i, "mpp")
        mpm_i = nodep.tile([128, PF], F32, tag="gmaxr", name="pf_rm", bufs=1)
        nc.scalar.activation(mpm_i[:], big2[:], AF.Relu, scale=-1.0)
        mpm = cross_max(mpm_i, "mpm")

        def ub_pos(ca, cb, name):
            t1 = tmul(ca, mpp, name + "_1")
            r1 = smalls.tile([128, 1], F32, name=name + "_r1")
            nc.scalar.activation(r1[:], t1[:], AF.Relu)
            t2 = tmul(cb, mpm, name + "_2")
            r2 = smalls.tile([128, 1], F32, name=name + "_r2")
            nc.scalar.activation(r2[:], t2[:], AF.Relu)
            o = smalls.tile([128, 1], F32, name=name)
            nc.vector.tensor_tensor(out=o[:], in0=r1[:], in1=r2[:], op=ALU.add)
            return o

        ub2 = smalls.tile([128, 1], F32, name="ub2")
        nc.vector.tensor_tensor(out=ub2[:], in0=ub_pos(A2, B2, "ubu2")[:],
                                in1=ub_pos(C2, D2, "ubv2")[:], op=ALU.add)
        gneg2 = lrelu_neg(ub2, "gneg2")

        # ------------- layer 2 edges -------------
        pgb = datp.tile([128, GSP], BF16, tag="base", name="pgb", bufs=1)
        for sg in range(NSIG):
            coff = (sg // 4) * Nshp + (sg % 4) * GS
            nc.scalar.dma_start(
                pgb[4 * sg:4 * sg + 4, 0:GS],
                p_fullb.ap()[:, coff:coff + GS].partition_broadcast(4))
        nc.vector.memset(pgb[:, GS:GSP], 0.0)
        psrc2 = sg_pass(pgb, "psrc2")

        rp = S("rp")
        nc.scalar.activation(rp[:], psrc2[:], AF.Relu)
        tmp2 = S("tmp2")
        nc.vector.tensor_scalar(out=tmp2[:], in0=psrc2[:], scalar1=nB2[:],
                                scalar2=None, op0=ALU.mult)
        u2 = S("u2")
        nc.vector.scalar_tensor_tensor(out=u2[:], in0=rp[:], scalar=A2B2[:],
                                       in1=tmp2[:], op0=ALU.mult, op1=ALU.add)
        epre2 = S("epre2")
        nc.vector.tensor_tensor(out=epre2[:], in0=u2[:], in1=pv2[:],
                                op=ALU.add)
        ae2 = S("ae2")
        nc.vector.scalar_tensor_tensor(out=ae2[:], in0=epre2[:], scalar=0.2,
                                       in1=epre2[:], op0=ALU.mult,
                                       op1=ALU.max)
        numer2 = S("numer2")
        nc.scalar.activation(numer2[:], ae2[:], AF.Exp, bias=gneg2[:])
        w21 = S("w21")
        nc.vector.tensor_tensor(out=w21[:], in0=numer2[:], in1=rp[:],
                                op=ALU.mult)
        w1b = S("w1b")
        nc.vector.tensor_tensor(out=w1b[:], in0=numer2[:], in1=psrc2[:],
                                op=ALU.mult)
        t0 = seg_scan(numer2, "t0")
        t1 = seg_scan(w21, "t1")
        t2 = seg_scan(w1b, "t2")
        bscatter(t0, 2, "t0")
        bscatter(t1, 3, "t1")
        bscatter(t2, 4, "t2")

        den2 = load_sums(2, "den2")
        Sp = load_sums(3, "Sp")
        Sraw = load_sums(4, "Sraw")
        # layer-2 self terms
        u2n = smalls.tile([128, W], F32, name="u2n", tag="nw", bufs=16)
        nc.vector.tensor_scalar(out=u2n[:], in0=rpn[:], scalar1=A2B2[:],
                                scalar2=None, op0=ALU.mult)
        u2n2 = smalls.tile([128, W], F32, name="u2n2", tag="nw", bufs=16)
        nc.vector.scalar_tensor_tensor(out=u2n2[:], in0=Pn[:], scalar=nB2[:],
                                       in1=u2n[:], op0=ALU.mult, op1=ALU.add)
        e2n = smalls.tile([128, W], F32, name="e2n", tag="nw", bufs=16)
        nc.vector.tensor_tensor(out=e2n[:], in0=u2n2[:], in1=v2sh[:],
                                op=ALU.add)
        sl2 = smalls.tile([128, W], F32, name="sl2", tag="nw", bufs=16)
        nc.vector.scalar_tensor_tensor(out=sl2[:], in0=e2n[:], scalar=0.2,
                                       in1=e2n[:], op0=ALU.mult, op1=ALU.max)
        selfn2 = smalls.tile([128, W], F32, name="selfn2", tag="nw", bufs=16)
        nc.scalar.activation(selfn2[:], sl2[:], AF.Exp, bias=gneg2[:])
        nc.vector.tensor_tensor(out=den2[:], in0=den2[:], in1=selfn2[:],
                                op=ALU.add)
        srp = smalls.tile([128, W], F32, name="srp", tag="nw", bufs=16)
        nc.vector.tensor_tensor(out=srp[:], in0=selfn2[:], in1=rpn[:],
                                op=ALU.mult)
        nc.vector.tensor_tensor(out=Sp[:], in0=Sp[:], in1=srp[:],
                                op=ALU.add)
        srw = smalls.tile([128, W], F32, name="srw", tag="nw", bufs=16)
        nc.vector.tensor_tensor(out=srw[:], in0=selfn2[:], in1=Pn[:],
                                op=ALU.mult)
        nc.vector.tensor_tensor(out=Sraw[:], in0=Sraw[:], in1=srw[:],
                                op=ALU.add)
        den2e = smalls.tile([128, W], F32, name="den2e", tag="nw", bufs=16)
        nc.vector.tensor_scalar(out=den2e[:], in0=den2[:], scalar1=1e-30,
                                scalar2=None, op0=ALU.add)
        rec2 = smalls.tile([128, W], F32, name="rec2", tag="nw", bufs=16)
        nc.vector.reciprocal(rec2[:], den2e[:])
        Rp2 = smalls.tile([128, W], F32, name="Rp2", tag="nw", bufs=16)
        nc.vector.tensor_tensor(out=Rp2[:], in0=Sp[:], in1=rec2[:],
                                op=ALU.mult)
        Smm = smalls.tile([128, W], F32, name="Smm", tag="nw", bufs=16)
        nc.vector.tensor_tensor(out=Smm[:], in0=Sp[:], in1=Sraw[:],
                                op=ALU.subtract)
        Rm = smalls.tile([128, W], F32, name="Rm", tag="nw", bufs=16)
        nc.vector.tensor_tensor(out=Rm[:], in0=Smm[:], in1=rec2[:],
                                op=ALU.mult)

        # y[d] = bl + sum_k relu(Rp*qp_k + Rm*qm_k + b2_k) * Wl_k
        yk = smalls.tile([128, W * 20], F32, name="yk", tag="yka", bufs=1)
        yk3 = yk[:].rearrange("p (w k) -> p w k", k=20)
        nc.vector.tensor_tensor(
            out=yk3,
            in0=Rp2[:].unsqueeze(2).broadcast_to([128, W, 20]),
            in1=qp[:].unsqueeze(1).broadcast_to([128, W, 20]), op=ALU.mult)
        yk2 = smalls.tile([128, W * 20], F32, name="yk2", tag="ykb", bufs=1)
        yk23 = yk2[:].rearrange("p (w k) -> p w k", k=20)
        nc.vector.tensor_tensor(
            out=yk23,
            in0=Rm[:].unsqueeze(2).broadcast_to([128, W, 20]),
            in1=qm[:].unsqueeze(1).broadcast_to([128, W, 20]), op=ALU.mult)
        nc.vector.tensor_tensor(out=yk[:], in0=yk[:], in1=yk2[:], op=ALU.add)
        nc.vector.tensor_tensor(
            out=yk3, in0=yk3,
            in1=b2t[:].unsqueeze(1).broadcast_to([128, W, 20]), op=ALU.add)
        nc.scalar.activation(yk[:], yk[:], AF.Relu)
        nc.vector.tensor_tensor(
            out=yk3, in0=yk3,
            in1=wlt[:].unsqueeze(1).broadcast_to([128, W, 20]), op=ALU.mult)
        yacc = smalls.tile([128, W], F32, name="yacc", tag="nw", bufs=16)
        nc.vector.tensor_reduce(yacc[:], yk3, AX.X, ALU.add)
        yf = smalls.tile([128, W], F32, name="yf", tag="nw", bufs=16)
        nc.vector.tensor_scalar(out=yf[:], in0=yacc[:], scalar1=blt[:],
                                scalar2=None, op0=ALU.add)
        nc.sync.dma_start(y_out.ap(), yf[:])

    nc.compile()
    return nc


def make_in_maps(pp, inputs):
    N, Nsh, Nshp = pp["N"], pp["Nsh"], pp["Nshp"]
    NF = -(-N // 128)
    x = np.asarray(inputs["x"], np.float32).reshape(-1)
    x_full = np.zeros(128 * NF, np.float32)
    x_full[:N] = x
    xb_full = x.astype(ml_dtypes.bfloat16)
    W2T = np.ascontiguousarray(np.asarray(inputs["W2"], np.float32).T)

    common = {
        "x_full": x_full[None, :],
        "xb_full": xb_full[None, :],
        "W1": np.asarray(inputs["W1"], np.float32).reshape(1, 20),
        "a_src1": np.asarray(inputs["a_src1"], np.float32).reshape(1, 20),
        "a_dst1": np.asarray(inputs["a_dst1"], np.float32).reshape(1, 20),
        "W2T": W2T.reshape(1, 400),
        "a_src2": np.asarray(inputs["a_src2"], np.float32).reshape(1, 20),
        "a_dst2": np.asarray(inputs["a_dst2"], np.float32).reshape(1, 20),
        "b2": np.asarray(inputs["b2"], np.float32).reshape(1, 20),
        "Wl": np.asarray(inputs["Wl"], np.float32).reshape(1, 20),
        "bl": np.asarray(inputs["bl"], np.float32).reshape(1, 1),
        "w4": pp["w4"],
    }
    maps = []
    for c in range(8):
        pc = pp["cores"][c]
        xs = np.zeros(Nshp, np.float32)
        xs[:Nsh] = x[c * Nsh:(c + 1) * Nsh]
        maps.append({
            **common,
            "x_shard": xs[None, :],
            "x_shardb": xs[None, :].astype(ml_dtypes.bfloat16),
            "runstart": pc["runstart"].astype(ml_dtypes.bfloat16),
            "impA": pc["impA"],
            "impB": pc["impB"],
            "bidx0": pc["bidx0"],
            "bidx1": pc["bidx1"],
            "sgA": pc["sgA"],
            "sgB": pc["sgB"],
        })
    return maps


def kernel(**inputs):
    x = np.asarray(inputs["x"], np.float32)
    N = x.shape[0]
    # device path assumes b1 == 0 (true for this problem) plus the layout
    # asserts in prep; fall back to numpy on anything unexpected.
    if np.any(np.asarray(inputs["b1"])) or N != N_NODES:
        return _kernel_numpy(**inputs)
    try:
        pp = prep(np.asarray(inputs["edge_index"]), N)
        nc = build(pp)
        maps = make_in_maps(pp, inputs)
    except Exception:
        return _kernel_numpy(**inputs)
    from concourse.bass_utils import run_bass_kernel_spmd
    res = run_bass_kernel_spmd(nc, maps, list(range(8)))
    Nsh = pp["Nsh"]
    y = np.zeros((N, 1), np.float32)
    for c in range(8):
        y[c * Nsh:(c + 1) * Nsh, 0] = res.results[c]["y"].reshape(-1)[:Nsh]
    return y


def _kernel_numpy(x, edge_index, W1, a_src1, a_dst1, b1, W2, a_src2, a_dst2,
                  b2, Wl, bl):
    def lr(v):
        return np.where(v > 0, v, 0.2 * v).astype(np.float32)

    def conv(h, src, dst, Wm, asrc, adst, b, n):
        hh = (h @ Wm).astype(np.float32)
        u, v = hh @ asrc, hh @ adst
        e = lr(u[src] + v[dst])
        m = np.full(n, -np.inf, np.float32)
        np.maximum.at(m, dst, e)
        ee = np.exp(e - m[dst]).astype(np.float32)
        den = np.bincount(dst, weights=ee, minlength=n).astype(np.float32)
        al = ee / (den[dst] + 1e-16)
        out = np.zeros((n, hh.shape[1]), np.float32)
        wh = hh[src] * al[:, None]
        for k in range(hh.shape[1]):
            out[:, k] = np.bincount(dst, weights=wh[:, k], minlength=n)
        return out + b

    n = x.shape[0]
    loop = np.arange(n, dtype=np.int64)
    src = np.concatenate([edge_index[0], loop])
    dst = np.concatenate([edge_index[1], loop])
    h = np.maximum(conv(np.asarray(x, np.float32), src, dst, W1, a_src1,
                        a_dst1, b1, n), 0)
    h = np.maximum(conv(h, src, dst, W2, a_src2, a_dst2, b2, n), 0)
    return (h @ Wl + bl).astype(np.float32)
